# revision 7
# baseline (speedup 1.0000x reference)
"""Trainium2 8-core kernel for nn_Attention_27530740367526.

Multi-head causal attention (B=2, S=2048, D=2048, H=16, HD=128, fp32) with
RoPE, sharded batch x head-group across 8 NeuronCores: core c handles batch
c//4 and heads [4*(c%4), 4*(c%4)+4).  Each core computes q/k/v projections
(+RoPE), attention for its 4 heads, and the slice of the wo projection those
heads feed — a partial [S, D] output.  The host sums the 4 partials per
batch (the row-parallel wo "all-reduce" is a host-side unshard).

Single fused pass: per 512-column sequence chunk (causal order) the kernel
projects q/k/v for all 4 local heads, runs attention for the chunk's queries
(head pairs interleaved so the PE always has two independent softmax chains),
and the previous chunk's wo projection (all 4 heads accumulated in PSUM, one
bf16 output write) drains into the attention's softmax-wait bubbles.

All matmul operands are bf16 (fast weight loads, half the DMA/SBUF), with
fp32 PSUM accumulation; the RoPE rotate-half runs as a f32r 128x128
permutation matmul on the PE.  Scores live in "transposed land" ([k, q] with
head-dim contraction) so softmax denominators come from an all-ones matmul
and PV/wo consume natural layouts with zero on-device transposes.  Diagonal
score tiles are narrowed to skip fully-masked columns.  Every DRAM tensor is
host-pre-tiled so each DMA descriptor is contiguous per partition.
"""

import sys

if "/opt/trn_rl_repo" not in sys.path:
    sys.path.insert(0, "/opt/trn_rl_repo")

from collections import deque

import numpy as np
import ml_dtypes

import concourse.bacc as bacc
import concourse.mybir as mybir
import concourse.tile as tile
from concourse.bass_utils import run_bass_kernel_spmd

F32 = mybir.dt.float32
F32R = mybir.dt.float32r
BF16 = mybir.dt.bfloat16
AF = mybir.ActivationFunctionType

N_HEADS = 16
N_CORES = 8
B, S, D = 2, 2048, 2048
HD = D // N_HEADS
H_LOC = N_HEADS // (N_CORES // B)  # 4 heads per core
SC = 512                           # seq chunk (matmul moving free dim)
P = 128
KO = D // P                        # 16 contraction subtiles for projections
NQC = S // SC                      # 4 q-chunks
NSUB = SC // P                     # 4 128-blocks per chunk
NST = S // P                       # 16 s-tiles
QKV_W = 3 * H_LOC * HD             # 1536 packed qkv columns
LOOKAHEAD = 3                      # scores-tile software pipeline depth


def _build_core_kernel():
    inv_sqrt_hd = 1.0 / float(np.sqrt(HD))

    nc = bacc.Bacc(None, target_bir_lowering=False)

    # host-pre-tiled inputs: every slice below is contiguous per partition
    xt_d = nc.dram_tensor("xt", [NQC, P, KO, SC], BF16, kind="ExternalInput")
    w_d = nc.dram_tensor("w", [KO, P, QKV_W], BF16, kind="ExternalInput")
    wo_d = nc.dram_tensor("wo", [P, H_LOC, D], BF16, kind="ExternalInput")
    cs_d = nc.dram_tensor("cs", [NQC, 2, P, SC], F32, kind="ExternalInput")
    pt_d = nc.dram_tensor("pt", [P, HD], F32R, kind="ExternalInput")
    ones_d = nc.dram_tensor("ones", [P, P], BF16, kind="ExternalInput")
    mask_d = nc.dram_tensor("mask", [P, NSUB, P], F32, kind="ExternalInput")
    y = nc.dram_tensor("y", [S, D], BF16, kind="ExternalOutput")

    with tile.TileContext(nc) as tc:
        with (
            tc.tile_pool(name="persist", bufs=1) as persist,
            tc.tile_pool(name="xa", bufs=2) as xa,
            tc.tile_pool(name="cs", bufs=2) as cspool,
            tc.tile_pool(name="scr", bufs=2) as scr,
            tc.tile_pool(name="exps", bufs=4) as expp,
            tc.tile_pool(name="outq", bufs=2) as outqp,
            tc.tile_pool(name="yo", bufs=4) as yop,
            tc.tile_pool(name="ps", bufs=3, space="PSUM") as cyc,
            tc.tile_pool(name="ops", bufs=2, space="PSUM") as ops,
            tc.tile_pool(name="dps", bufs=2, space="PSUM") as dps,
            tc.tile_pool(name="yps", bufs=1, space="PSUM") as yps,
        ):
            # small persistent constants (scalar queue, ahead of big loads)
            pt_sb = persist.tile([P, HD], F32R)
            nc.scalar.dma_start(pt_sb[:], pt_d[:])
            ones_sb = persist.tile([P, P], BF16)
            nc.scalar.dma_start(ones_sb[:], ones_d[:])
            mask_sb = persist.tile([P, NSUB, P], F32)
            nc.scalar.dma_start(mask_sb[:], mask_d[:])

            # per-head-pair persistent k/v for the whole sequence
            kT_sb = persist.tile([P, H_LOC, S], BF16)
            v_sb = persist.tile([P, NST, H_LOC * HD], BF16)
            w_sb = persist.tile([P, KO, QKV_W], BF16)
            wo_sb = persist.tile([P, H_LOC, D], BF16)

            def load_chunk(sc):
                xt = xa.tile([P, KO, SC], BF16, tag="xt")
                for kg in range(4):  # 4-ko groups: 4 KB contiguous rows
                    nc.sync.dma_start(
                        xt[:, 4 * kg : 4 * kg + 4], xt_d[sc, :, 4 * kg : 4 * kg + 4]
                    )
                cos_t = cspool.tile([P, SC], F32, tag="cos")
                sin_t = cspool.tile([P, SC], F32, tag="sin")
                nc.sync.dma_start(cos_t[:], cs_d[sc, 0])
                nc.sync.dma_start(sin_t[:], cs_d[sc, 1])
                return xt, cos_t, sin_t

            # x chunk 0 queued ahead of the (large) weight loads so the PE
            # can start within a few us; the first projection group's weight
            # columns (h0 q+k) ride the sync queue right behind it, the rest
            # stream on the scalar queue
            preloaded = load_chunk(0)
            for ko in range(KO):
                nc.sync.dma_start(w_sb[:, ko, : 2 * HD], w_d[ko, :, : 2 * HD])
            for ko in range(KO):
                nc.scalar.dma_start(
                    w_sb[:, ko, 2 * HD :], w_d[ko, :, 2 * HD :]
                )
            for h in range(H_LOC):
                nc.scalar.dma_start(wo_sb[:, h], wo_d[:, h])

            # dummy matmuls: trip the PE HAM clock-gate to full rate and
            # cover the initial weight-stream latency with filler activity
            for wu in range(40):
                wps = cyc.tile([P, SC], F32, tag="ps")
                nc.tensor.matmul(
                    wps[:, :P], ones_sb[:], ones_sb[:], skip_group_check=True
                )

            def project_chunk(sc, loaded):
                ssl = slice(sc * SC, (sc + 1) * SC)
                xt, cos_t, sin_t = loaded
                qT_c = outqp.tile([P, H_LOC, SC], BF16, tag="qTc")

                for h in range(H_LOC):
                    for t in range(2):  # 0=q, 1=k
                        wcols = slice((2 * h + t) * HD, (2 * h + t + 1) * HD)
                        ps = cyc.tile([P, SC], F32, tag="ps")
                        for ko in range(KO):
                            nc.tensor.matmul(
                                ps[:],
                                w_sb[:, ko, wcols],
                                xt[:, ko],
                                start=(ko == 0),
                                stop=(ko == KO - 1),
                            )
                        plain = scr.tile([P, SC], F32R, tag="plain")
                        nc.scalar.copy(plain[:], ps[:])
                        rot = cyc.tile([P, SC], F32, tag="ps")
                        nc.tensor.matmul(rot[:], pt_sb[:], plain[:])
                        dst = qT_c[:, h, :] if t == 0 else kT_sb[:, h, ssl]
                        # rope: dst = plain*cos + rot*sin
                        pc = scr.tile([P, SC], F32, tag="pc")
                        nc.gpsimd.tensor_mul(pc[:], plain[:], cos_t[:])
                        tmp2 = scr.tile([P, SC], F32, tag="tmp2")
                        nc.vector.tensor_mul(tmp2[:], rot[:], sin_t[:])
                        nc.vector.tensor_add(dst, pc[:], tmp2[:])

                for sti in range(NSUB):
                    st = sc * NSUB + sti
                    lsl = slice(sti * P, (sti + 1) * P)
                    psv = cyc.tile([P, H_LOC * HD], F32, tag="ps")
                    for ko in range(KO):
                        nc.tensor.matmul(
                            psv[:],
                            xt[:, ko, lsl],
                            w_sb[:, ko, 2 * H_LOC * HD :],
                            start=(ko == 0),
                            stop=(ko == KO - 1),
                        )
                    nc.scalar.copy(v_sb[:, st, :], psv[:])
                return qT_c

            def attend_pair(qc, qT_c, hp, outT_qc, fillers):
                """Attention for query chunk qc, heads (2hp, 2hp+1)
                interleaved per k-block (so the PE always has two
                independent softmax chains in flight), writing normalized
                outT [hd, q] slices.  `fillers` is a deque of independent
                PE-work closures drained into the pipeline's tail bubbles.
                Diagonal k-blocks are narrowed to their live q columns."""
                nkb = (qc + 1) * NSUB
                qt = {}
                o_ps = {}
                d_ps = {}
                for hl in range(2):
                    h = 2 * hp + hl
                    qt[hl] = qT_c[:, h, :]
                    o_ps[hl] = ops.tile([P, SC], F32, tag="o", name=f"o_ps{hl}")
                    d_ps[hl] = dps.tile([P, SC], F32, tag="d", name=f"d_ps{hl}")
                stile = {}

                def q0(kb):
                    # first live q column for k-block kb (causal narrowing)
                    j = kb - qc * NSUB
                    return j * P if j > 0 else 0

                def emit_scores(kb, hl):
                    h = 2 * hp + hl
                    c0 = q0(kb)
                    t_ = cyc.tile([P, SC], F32, tag="ps")
                    nc.tensor.matmul(
                        t_[:, c0:],
                        kT_sb[:, h, kb * P : (kb + 1) * P],
                        qt[hl][:, c0:],
                        skip_group_check=True,
                    )
                    j = kb - qc * NSUB
                    if j >= 0:
                        # triangular boundary block only
                        nc.vector.tensor_add(
                            t_[:, c0 : c0 + P],
                            t_[:, c0 : c0 + P],
                            mask_sb[:, j, :],
                        )
                    stile[(kb, hl)] = t_

                seq = [(kb, hl) for kb in range(nkb) for hl in range(2)]
                for kb, hl in seq[:LOOKAHEAD]:
                    emit_scores(kb, hl)
                for i, (kb, hl) in enumerate(seq):
                    c0 = q0(kb)
                    h = 2 * hp + hl
                    e = expp.tile([P, SC], BF16, tag="e")
                    nc.scalar.activation(
                        e[:, c0:], stile.pop((kb, hl))[:, c0:], AF.Exp,
                        scale=inv_sqrt_hd,
                    )
                    nc.tensor.matmul(
                        o_ps[hl][:, c0:],
                        v_sb[:, kb, h * HD : (h + 1) * HD],
                        e[:, c0:],
                        start=(kb == 0),
                        stop=(kb == nkb - 1),
                        skip_group_check=True,
                    )
                    nc.tensor.matmul(
                        d_ps[hl][:, c0:],
                        ones_sb[:],
                        e[:, c0:],
                        start=(kb == 0),
                        stop=(kb == nkb - 1),
                        skip_group_check=True,
                    )
                    if i + LOOKAHEAD < len(seq):
                        emit_scores(*seq[i + LOOKAHEAD])
                        if fillers and i % 2 == 1:
                            fillers.popleft()()
                    elif fillers:
                        fillers.popleft()()
                for hl in range(2):
                    h = 2 * hp + hl
                    recip = scr.tile([P, SC], F32, tag="recip")
                    nc.vector.reciprocal_approx_fast(recip[:], d_ps[hl][:])
                    nc.vector.tensor_mul(
                        outT_qc[:, h, :], o_ps[hl][:], recip[:]
                    )

            def make_out_fillers(qc, outT_qc, tail=False):
                """One closure per (s-tile, d-chunk) block of the wo
                projection for query chunk qc: 4 accumulating matmuls (all
                local heads), a PSUM->SBUF bf16 copy, and the output DMA.
                Tail blocks (after the last attend) ping-pong across the
                now-idle attention PSUM banks and both copy engines."""
                work = []
                for sti in range(NSUB):
                    st = qc * NSUB + sti
                    stsl = slice(sti * P, (sti + 1) * P)
                    for dc in range(D // SC):
                        dsl = slice(dc * SC, (dc + 1) * SC)
                        bi = sti * (D // SC) + dc

                        def blk(st=st, stsl=stsl, dsl=dsl, bi=bi):
                            if tail:
                                pool, tag = [
                                    (yps, "y"), (dps, "d"), (ops, "o")
                                ][bi % 3]
                                y_ps = pool.tile([P, SC], F32, tag=tag)
                            else:
                                y_ps = yps.tile([P, SC], F32, tag="y")
                            for h in range(H_LOC):
                                nc.tensor.matmul(
                                    y_ps[:],
                                    outT_qc[:, h, stsl],
                                    wo_sb[:, h, dsl],
                                    start=(h == 0),
                                    stop=(h == H_LOC - 1),
                                )
                            y_sb = yop.tile([P, SC], BF16, tag="ysb")
                            if tail and bi % 2 == 1:
                                nc.scalar.copy(y_sb[:], y_ps[:])
                            else:
                                nc.vector.tensor_copy(y_sb[:], y_ps[:])
                            nc.sync.dma_start(
                                y[st * P : (st + 1) * P, dsl], y_sb[:]
                            )

                        work.append(blk)
                return work

            pending = deque()
            for sc in range(NQC):
                loaded = preloaded if sc == 0 else load_chunk(sc)
                preloaded = None
                qT_c = project_chunk(sc, loaded)
                outT_qc = outqp.tile([P, H_LOC, SC], BF16, tag="outq")
                for hp in range(2):
                    attend_pair(sc, qT_c, hp, outT_qc, pending)
                pending.extend(
                    make_out_fillers(sc, outT_qc, tail=(sc == NQC - 1))
                )
            while pending:
                pending.popleft()()

    nc.compile()
    return nc


_NC_CACHE = {}


def _get_nc():
    if "nc" not in _NC_CACHE:
        _NC_CACHE["nc"] = _build_core_kernel()
    return _NC_CACHE["nc"]


def _rope_perm_T() -> np.ndarray:
    # rotate_half as a matrix: (P_rh @ q)[d] = -q[d+HD/2] for d < HD/2,
    # q[d-HD/2] otherwise.  Returns P_rh.T for use as matmul lhsT.
    P_rh = np.zeros((HD, HD), dtype=np.float32)
    half = HD // 2
    for i in range(half):
        P_rh[i, half + i] = -1.0
        P_rh[half + i, i] = 1.0
    return np.ascontiguousarray(P_rh.T)


def _is_causal(m: np.ndarray) -> bool:
    tril = np.tril(np.ones((S, S), dtype=bool))
    if not np.all(m[tril] == 0.0):
        return False
    upper = m[~tril]
    return bool(upper.size == 0 or np.all(upper <= -1.0e8))


def _bf16(x: np.ndarray) -> np.ndarray:
    return np.ascontiguousarray(x).astype(ml_dtypes.bfloat16)


def _reference_numpy(x, cos, sin, mask, wq, wk, wv, wo):
    # generic-mask fallback (never hit for the causal reference mask)
    def rot_half(t):
        t1, t2 = np.split(t, 2, axis=-1)
        return np.concatenate((-t2, t1), axis=-1)

    H = N_HEADS
    q = (x @ wq.T).reshape(B, S, H, HD).transpose(0, 2, 1, 3)
    k = (x @ wk.T).reshape(B, S, H, HD).transpose(0, 2, 1, 3)
    v = (x @ wv.T).reshape(B, S, H, HD).transpose(0, 2, 1, 3)
    c = cos[None, None]
    s = sin[None, None]
    q = q * c + rot_half(q) * s
    k = k * c + rot_half(k) * s
    scores = np.einsum("bhqd,bhkd->bhqk", q, k) / np.sqrt(np.float32(HD))
    scores = scores + mask
    scores -= scores.max(axis=-1, keepdims=True)
    p = np.exp(scores)
    p /= p.sum(axis=-1, keepdims=True)
    out = np.einsum("bhqk,bhkd->bhqd", p, v)
    out = out.transpose(0, 2, 1, 3).reshape(B, S, D)
    return (out @ wo.T).astype(np.float32)


# module-level: results of the last traced run (for test harnesses)
last_exec_time_ns = None
last_profile_json = None


def kernel(x, cos, sin, mask, wq, wk, wv, wo, _trace=False):
    x = np.asarray(x, dtype=np.float32)
    cos = np.asarray(cos, dtype=np.float32)
    sin = np.asarray(sin, dtype=np.float32)
    mask = np.asarray(mask, dtype=np.float32)
    wq = np.asarray(wq, dtype=np.float32)
    wk = np.asarray(wk, dtype=np.float32)
    wv = np.asarray(wv, dtype=np.float32)
    wo = np.asarray(wo, dtype=np.float32)

    m2d = mask.reshape(S, S)
    if not _is_causal(m2d):
        return _reference_numpy(x, cos, sin, mask, wq, wk, wv, wo)
    nc = _get_nc()

    scale = np.float32(np.sqrt(HD))
    # [k, q] transposed causal boundary blocks: mask_h[ki, j, q_local]
    mt = np.ascontiguousarray((m2d[:SC, :SC] * scale).T).reshape(NSUB, P, NSUB, P)
    mask_h = np.ascontiguousarray(
        np.stack([mt[j, :, j, :] for j in range(NSUB)], axis=1)
    )
    # cos/sin chunk-tiled: cs[sc, {cos,sin}, hd, s_local]
    cs = np.stack([cos.T, sin.T], axis=0).reshape(2, HD, NQC, SC)
    cs = np.ascontiguousarray(cs.transpose(2, 0, 1, 3), dtype=np.float32)
    ptT = _rope_perm_T()
    ones = np.ones((P, P), dtype=np.float32)

    # x chunk-tiled: xt[sc, ki, ko, s_local]
    xts = []
    for b in range(B):
        xT = x[b].T.reshape(KO, P, NQC, SC)
        xts.append(_bf16(xT.transpose(2, 1, 0, 3)))

    in_maps = []
    for c in range(N_CORES):
        b = c // (N_CORES // B)
        hg = c % (N_CORES // B)
        # qkv packed per head: [q_h0|k_h0|...|q_h3|k_h3|v_h0..v_h3],
        # laid out [ko, ki, col]
        cols = []
        for h in range(H_LOC):
            hh = hg * H_LOC + h
            cols.append(wq[hh * HD : (hh + 1) * HD].T)
            cols.append(wk[hh * HD : (hh + 1) * HD].T)
        for h in range(H_LOC):
            hh = hg * H_LOC + h
            cols.append(wv[hh * HD : (hh + 1) * HD].T)
        wpack = np.concatenate(cols, axis=1)  # [D, 1536]
        wpack = np.ascontiguousarray(wpack.reshape(KO, P, QKV_W))
        # wo rows for this head group, laid out [ki, h, d]
        rows = slice(hg * H_LOC * HD, (hg + 1) * H_LOC * HD)
        wot = wo[:, rows].T.reshape(H_LOC, P, D)
        wot = np.ascontiguousarray(wot.transpose(1, 0, 2))
        in_maps.append(
            {
                "xt": xts[b],
                "w": _bf16(wpack),
                "wo": _bf16(wot),
                "cs": cs,
                "pt": ptT,
                "ones": _bf16(ones),
                "mask": mask_h,
            }
        )

    kw = {}
    if _trace:
        kw = dict(trace=True)
    res = run_bass_kernel_spmd(
        nc, in_maps, core_ids=list(range(N_CORES)), **kw
    )
    global last_exec_time_ns, last_profile_json
    last_exec_time_ns = res.exec_time_ns
    last_profile_json = res.profile_json

    out = np.empty((B, S, D), dtype=np.float32)
    gs = N_CORES // B
    for b in range(B):
        acc = res.results[b * gs]["y"].astype(np.float32)
        for g in range(1, gs):
            acc += res.results[b * gs + g]["y"].astype(np.float32)
        out[b] = acc
    return out


# revision 16
# speedup vs baseline: 1.1292x; 1.1292x over previous
"""Trainium2 8-core kernel for nn_Attention_27530740367526.

Multi-head causal attention (B=2, S=2048, D=2048, H=16, HD=128, fp32) with
RoPE, sharded batch x head-group across 8 NeuronCores: core c handles batch
c//4 and heads [4*(c%4), 4*(c%4)+4).  Each core computes q/k/v projections
(+RoPE), attention for its 4 heads, and the slice of the wo projection those
heads feed — a partial [S, D] output.  The host sums the 4 partials per
batch (the row-parallel wo "all-reduce" is a host-side unshard).

Single fused pass: per 512-column sequence chunk (causal order) the kernel
projects q/k/v for all 4 local heads, runs attention for the chunk's queries
(head pairs interleaved so the PE always has two independent softmax chains),
and the previous chunk's wo projection (all 4 heads accumulated in PSUM, one
bf16 output write) drains into the attention's softmax-wait bubbles.

All matmul operands are bf16 (fast weight loads, half the DMA/SBUF), with
fp32 PSUM accumulation; the RoPE rotate-half runs as a f32r 128x128
permutation matmul on the PE.  Scores live in "transposed land" ([k, q] with
head-dim contraction) so softmax denominators come from an all-ones matmul
and PV/wo consume natural layouts with zero on-device transposes.  Diagonal
score tiles are narrowed to skip fully-masked columns.  Every DRAM tensor is
host-pre-tiled so each DMA descriptor is contiguous per partition.
"""

import sys

if "/opt/trn_rl_repo" not in sys.path:
    sys.path.insert(0, "/opt/trn_rl_repo")

from collections import deque

import numpy as np
import ml_dtypes

import concourse.bacc as bacc
import concourse.mybir as mybir
import concourse.tile as tile
from concourse.bass_utils import run_bass_kernel_spmd

F32 = mybir.dt.float32
F32R = mybir.dt.float32r
BF16 = mybir.dt.bfloat16
AF = mybir.ActivationFunctionType

N_HEADS = 16
N_CORES = 8
B, S, D = 2, 2048, 2048
HD = D // N_HEADS
H_LOC = N_HEADS // (N_CORES // B)  # 4 heads per core
SC = 512                           # seq chunk (matmul moving free dim)
P = 128
KO = D // P                        # 16 contraction subtiles for projections
NQC = S // SC                      # 4 q-chunks
NSUB = SC // P                     # 4 128-blocks per chunk
NST = S // P                       # 16 s-tiles
QKV_W = 3 * H_LOC * HD             # 1536 packed qkv columns
LOOKAHEAD = 3                      # scores-tile software pipeline depth


def _build_core_kernel():
    inv_sqrt_hd = 1.0 / float(np.sqrt(HD))

    nc = bacc.Bacc(None, target_bir_lowering=False)

    # host-pre-tiled inputs: every slice below is contiguous per partition
    xt_d = nc.dram_tensor("xt", [NQC, P, KO, SC], BF16, kind="ExternalInput")
    wqk_d = nc.dram_tensor(
        "wqk", [KO, P, 2 * H_LOC * HD], BF16, kind="ExternalInput"
    )
    wv_d = nc.dram_tensor(
        "wv", [P, KO, H_LOC * HD], BF16, kind="ExternalInput"
    )
    wo_d = nc.dram_tensor("wo", [P, H_LOC, D], BF16, kind="ExternalInput")
    cs_d = nc.dram_tensor("cs", [NQC, 2, P, SC], F32, kind="ExternalInput")
    pt_d = nc.dram_tensor("pt", [P, HD], F32R, kind="ExternalInput")
    ones_d = nc.dram_tensor("ones", [P, P], BF16, kind="ExternalInput")
    mask_d = nc.dram_tensor("mask", [P, NSUB, P], F32, kind="ExternalInput")
    y = nc.dram_tensor("y", [S, D], BF16, kind="ExternalOutput")

    with tile.TileContext(nc) as tc:
        with (
            tc.tile_pool(name="persist", bufs=1) as persist,
            tc.tile_pool(name="xa", bufs=2) as xa,
            tc.tile_pool(name="cs", bufs=2) as cspool,
            tc.tile_pool(name="scr", bufs=2) as scr,
            tc.tile_pool(name="exps", bufs=4) as expp,
            tc.tile_pool(name="outq", bufs=2) as outqp,
            tc.tile_pool(name="yo", bufs=4) as yop,
            tc.tile_pool(name="ps", bufs=3, space="PSUM") as cyc,
            tc.tile_pool(name="ops", bufs=2, space="PSUM") as ops,
            tc.tile_pool(name="dps", bufs=2, space="PSUM") as dps,
            tc.tile_pool(name="yps", bufs=1, space="PSUM") as yps,
        ):
            # small persistent constants (scalar queue, ahead of big loads)
            pt_sb = persist.tile([P, HD], F32R)
            nc.scalar.dma_start(pt_sb[:], pt_d[:])
            ones_sb = persist.tile([P, P], BF16)
            nc.scalar.dma_start(ones_sb[:], ones_d[:])
            mask_sb = persist.tile([P, NSUB, P], F32)
            nc.scalar.dma_start(mask_sb[:], mask_d[:])

            # per-head-pair persistent k/v for the whole sequence
            kT_sb = persist.tile([P, H_LOC, S], BF16)
            v_sb = persist.tile([P, NST, H_LOC * HD], BF16)
            wqk_sb = persist.tile([P, KO, 2 * H_LOC * HD], BF16)
            wv_sb = persist.tile([P, KO, H_LOC * HD], BF16)
            wo_sb = persist.tile([P, H_LOC, D], BF16)

            def load_chunk(sc):
                # prefetched a chunk ahead: one fully-contiguous descriptor
                # on the scalar queue, which is idle of DMA work by then
                xt = xa.tile([P, KO, SC], BF16, tag="xt")
                nc.scalar.dma_start(xt[:], xt_d[sc])
                cos_t = cspool.tile([P, SC], F32, tag="cos")
                sin_t = cspool.tile([P, SC], F32, tag="sin")
                nc.scalar.dma_start(cos_t[:], cs_d[sc, 0])
                nc.scalar.dma_start(sin_t[:], cs_d[sc, 1])
                return xt, cos_t, sin_t

            # startup: the first projection group is gated by (xt0[ko],
            # wqk[ko]) pairs, so interleave those on the sync queue; the
            # v-weights (first needed ~25us in) and wo (first needed at the
            # first wo fillers) stream on the scalar queue
            xt0 = xa.tile([P, KO, SC], BF16, tag="xt")
            for kg in range(4):
                nc.sync.dma_start(
                    xt0[:, 4 * kg : 4 * kg + 4],
                    xt_d[0, :, 4 * kg : 4 * kg + 4],
                )
                for ko in range(4 * kg, 4 * kg + 4):
                    nc.sync.dma_start(wqk_sb[:, ko], wqk_d[ko])
            cos_0 = cspool.tile([P, SC], F32, tag="cos")
            sin_0 = cspool.tile([P, SC], F32, tag="sin")
            nc.sync.dma_start(cos_0[:], cs_d[0, 0])
            nc.sync.dma_start(sin_0[:], cs_d[0, 1])
            preloaded = (xt0, cos_0, sin_0)
            nc.scalar.dma_start(wv_sb[:], wv_d[:])
            for h in range(H_LOC):
                nc.scalar.dma_start(wo_sb[:, h], wo_d[:, h])

            # dummy matmuls: trip the PE HAM clock-gate to full rate and
            # cover the initial weight-stream latency with filler activity.
            # They write the (startup-idle) y PSUM bank; the bufs=1 WAW
            # chain serializes them on the PE only.
            def warmup(n):
                for wu in range(n):
                    wps = yps.tile([P, SC], F32, tag="y")
                    nc.tensor.matmul(
                        wps[:, :P], ones_sb[:], ones_sb[:],
                        skip_group_check=True,
                    )

            warmup(16)

            def project_chunk(sc, loaded):
                ssl = slice(sc * SC, (sc + 1) * SC)
                xt, cos_t, sin_t = loaded
                qT_c = outqp.tile([P, H_LOC, SC], BF16, tag="qTc")

                for h in range(H_LOC):
                    for t in range(2):  # 0=q, 1=k
                        wcols = slice((2 * h + t) * HD, (2 * h + t + 1) * HD)
                        ps = cyc.tile([P, SC], F32, tag="ps")
                        for ko in range(KO):
                            nc.tensor.matmul(
                                ps[:],
                                wqk_sb[:, ko, wcols],
                                xt[:, ko],
                                start=(ko == 0),
                                stop=(ko == KO - 1),
                            )
                            if sc == 0 and h == 0 and t == 0:
                                # first group is DMA-gated: keep the PE (and
                                # its clock-gate) busy while weights stream
                                warmup(10)
                        plain = scr.tile([P, SC], F32R, tag="plain")
                        nc.scalar.copy(plain[:], ps[:])
                        rot = cyc.tile([P, SC], F32, tag="ps")
                        nc.tensor.matmul(rot[:], pt_sb[:], plain[:])
                        dst = qT_c[:, h, :] if t == 0 else kT_sb[:, h, ssl]
                        # rope: dst = plain*cos + rot*sin
                        pc = scr.tile([P, SC], F32, tag="pc")
                        nc.gpsimd.tensor_mul(pc[:], plain[:], cos_t[:])
                        tmp2 = scr.tile([P, SC], F32, tag="tmp2")
                        nc.vector.tensor_mul(tmp2[:], rot[:], sin_t[:])
                        nc.vector.tensor_add(dst, pc[:], tmp2[:])

                for sti in range(NSUB):
                    st = sc * NSUB + sti
                    lsl = slice(sti * P, (sti + 1) * P)
                    psv = cyc.tile([P, H_LOC * HD], F32, tag="ps")
                    for ko in range(KO):
                        nc.tensor.matmul(
                            psv[:],
                            xt[:, ko, lsl],
                            wv_sb[:, ko],
                            start=(ko == 0),
                            stop=(ko == KO - 1),
                        )
                    nc.scalar.copy(v_sb[:, st, :], psv[:])
                return qT_c

            def attend_pair(qc, qT_c, hp, outT_qc, fillers):
                """Attention for query chunk qc, heads (2hp, 2hp+1)
                interleaved per k-block (so the PE always has two
                independent softmax chains in flight), writing normalized
                outT [hd, q] slices.  `fillers` is a deque of independent
                PE-work closures drained into the pipeline's tail bubbles.
                Diagonal k-blocks are narrowed to their live q columns."""
                nkb = (qc + 1) * NSUB
                qt = {}
                o_ps = {}
                d_ps = {}
                for hl in range(2):
                    h = 2 * hp + hl
                    qt[hl] = qT_c[:, h, :]
                    o_ps[hl] = ops.tile([P, SC], F32, tag="o", name=f"o_ps{hl}")
                    d_ps[hl] = dps.tile([P, SC], F32, tag="d", name=f"d_ps{hl}")
                stile = {}

                def q0(kb):
                    # first live q column for k-block kb (causal narrowing)
                    j = kb - qc * NSUB
                    return j * P if j > 0 else 0

                def emit_scores(kb, hl):
                    h = 2 * hp + hl
                    c0 = q0(kb)
                    t_ = cyc.tile([P, SC], F32, tag="ps")
                    nc.tensor.matmul(
                        t_[:, c0:],
                        kT_sb[:, h, kb * P : (kb + 1) * P],
                        qt[hl][:, c0:],
                        skip_group_check=True,
                    )
                    j = kb - qc * NSUB
                    if j >= 0:
                        # triangular boundary block only
                        nc.vector.tensor_add(
                            t_[:, c0 : c0 + P],
                            t_[:, c0 : c0 + P],
                            mask_sb[:, j, :],
                        )
                    stile[(kb, hl)] = t_

                seq = [(kb, hl) for kb in range(nkb) for hl in range(2)]
                for kb, hl in seq[:LOOKAHEAD]:
                    emit_scores(kb, hl)
                for i, (kb, hl) in enumerate(seq):
                    c0 = q0(kb)
                    h = 2 * hp + hl
                    e = expp.tile([P, SC], BF16, tag="e")
                    nc.scalar.activation(
                        e[:, c0:], stile.pop((kb, hl))[:, c0:], AF.Exp,
                        scale=inv_sqrt_hd,
                    )
                    nc.tensor.matmul(
                        o_ps[hl][:, c0:],
                        v_sb[:, kb, h * HD : (h + 1) * HD],
                        e[:, c0:],
                        start=(kb == 0),
                        stop=(kb == nkb - 1),
                        skip_group_check=True,
                    )
                    nc.tensor.matmul(
                        d_ps[hl][:, c0:],
                        ones_sb[:],
                        e[:, c0:],
                        start=(kb == 0),
                        stop=(kb == nkb - 1),
                        skip_group_check=True,
                    )
                    if i + LOOKAHEAD < len(seq):
                        emit_scores(*seq[i + LOOKAHEAD])
                        if fillers and i % 2 == 1:
                            fillers.popleft()()
                    elif fillers:
                        fillers.popleft()()
                for hl in range(2):
                    h = 2 * hp + hl
                    recip = scr.tile([P, SC], F32, tag="recip")
                    nc.vector.reciprocal_approx_fast(recip[:], d_ps[hl][:])
                    nc.vector.tensor_mul(
                        outT_qc[:, h, :], o_ps[hl][:], recip[:]
                    )

            def make_out_fillers(qc, outT_qc, tail=False):
                """One closure per (s-tile, d-chunk) block of the wo
                projection for query chunk qc: 4 accumulating matmuls (all
                local heads), a PSUM->SBUF bf16 copy, and the output DMA.
                Tail blocks (after the last attend) ping-pong across the
                now-idle attention PSUM banks and both copy engines."""
                work = []
                for sti in range(NSUB):
                    st = qc * NSUB + sti
                    stsl = slice(sti * P, (sti + 1) * P)
                    for dc in range(D // SC):
                        dsl = slice(dc * SC, (dc + 1) * SC)
                        bi = sti * (D // SC) + dc

                        def blk(st=st, stsl=stsl, dsl=dsl, bi=bi):
                            if tail:
                                pool, tag = [
                                    (yps, "y"), (dps, "d"), (ops, "o")
                                ][bi % 3]
                                y_ps = pool.tile([P, SC], F32, tag=tag)
                            else:
                                y_ps = yps.tile([P, SC], F32, tag="y")
                            for h in range(H_LOC):
                                nc.tensor.matmul(
                                    y_ps[:],
                                    outT_qc[:, h, stsl],
                                    wo_sb[:, h, dsl],
                                    start=(h == 0),
                                    stop=(h == H_LOC - 1),
                                )
                            y_sb = yop.tile([P, SC], BF16, tag="ysb")
                            if tail and bi % 2 == 1:
                                nc.scalar.copy(y_sb[:], y_ps[:])
                            else:
                                nc.vector.tensor_copy(y_sb[:], y_ps[:])
                            nc.sync.dma_start(
                                y[st * P : (st + 1) * P, dsl], y_sb[:]
                            )

                        work.append(blk)
                return work

            pending = deque()
            for sc in range(NQC):
                loaded = preloaded if sc == 0 else load_chunk(sc)
                preloaded = None
                qT_c = project_chunk(sc, loaded)
                outT_qc = outqp.tile([P, H_LOC, SC], BF16, tag="outq")
                for hp in range(2):
                    attend_pair(sc, qT_c, hp, outT_qc, pending)
                pending.extend(
                    make_out_fillers(sc, outT_qc, tail=(sc == NQC - 1))
                )
            while pending:
                pending.popleft()()

    nc.compile()
    return nc


_NC_CACHE = {}


def _get_nc():
    if "nc" not in _NC_CACHE:
        _NC_CACHE["nc"] = _build_core_kernel()
    return _NC_CACHE["nc"]


def _rope_perm_T() -> np.ndarray:
    # rotate_half as a matrix: (P_rh @ q)[d] = -q[d+HD/2] for d < HD/2,
    # q[d-HD/2] otherwise.  Returns P_rh.T for use as matmul lhsT.
    P_rh = np.zeros((HD, HD), dtype=np.float32)
    half = HD // 2
    for i in range(half):
        P_rh[i, half + i] = -1.0
        P_rh[half + i, i] = 1.0
    return np.ascontiguousarray(P_rh.T)


def _is_causal(m: np.ndarray) -> bool:
    tril = np.tril(np.ones((S, S), dtype=bool))
    if not np.all(m[tril] == 0.0):
        return False
    upper = m[~tril]
    return bool(upper.size == 0 or np.all(upper <= -1.0e8))


def _bf16(x: np.ndarray) -> np.ndarray:
    return np.ascontiguousarray(x).astype(ml_dtypes.bfloat16)


def _reference_numpy(x, cos, sin, mask, wq, wk, wv, wo):
    # generic-mask fallback (never hit for the causal reference mask)
    def rot_half(t):
        t1, t2 = np.split(t, 2, axis=-1)
        return np.concatenate((-t2, t1), axis=-1)

    H = N_HEADS
    q = (x @ wq.T).reshape(B, S, H, HD).transpose(0, 2, 1, 3)
    k = (x @ wk.T).reshape(B, S, H, HD).transpose(0, 2, 1, 3)
    v = (x @ wv.T).reshape(B, S, H, HD).transpose(0, 2, 1, 3)
    c = cos[None, None]
    s = sin[None, None]
    q = q * c + rot_half(q) * s
    k = k * c + rot_half(k) * s
    scores = np.einsum("bhqd,bhkd->bhqk", q, k) / np.sqrt(np.float32(HD))
    scores = scores + mask
    scores -= scores.max(axis=-1, keepdims=True)
    p = np.exp(scores)
    p /= p.sum(axis=-1, keepdims=True)
    out = np.einsum("bhqk,bhkd->bhqd", p, v)
    out = out.transpose(0, 2, 1, 3).reshape(B, S, D)
    return (out @ wo.T).astype(np.float32)


# module-level: results of the last traced run (for test harnesses)
last_exec_time_ns = None
last_profile_json = None


def kernel(x, cos, sin, mask, wq, wk, wv, wo, _trace=False):
    x = np.asarray(x, dtype=np.float32)
    cos = np.asarray(cos, dtype=np.float32)
    sin = np.asarray(sin, dtype=np.float32)
    mask = np.asarray(mask, dtype=np.float32)
    wq = np.asarray(wq, dtype=np.float32)
    wk = np.asarray(wk, dtype=np.float32)
    wv = np.asarray(wv, dtype=np.float32)
    wo = np.asarray(wo, dtype=np.float32)

    m2d = mask.reshape(S, S)
    if not _is_causal(m2d):
        return _reference_numpy(x, cos, sin, mask, wq, wk, wv, wo)
    nc = _get_nc()

    scale = np.float32(np.sqrt(HD))
    # [k, q] transposed causal boundary blocks: mask_h[ki, j, q_local]
    mt = np.ascontiguousarray((m2d[:SC, :SC] * scale).T).reshape(NSUB, P, NSUB, P)
    mask_h = np.ascontiguousarray(
        np.stack([mt[j, :, j, :] for j in range(NSUB)], axis=1)
    )
    # cos/sin chunk-tiled: cs[sc, {cos,sin}, hd, s_local]
    cs = np.stack([cos.T, sin.T], axis=0).reshape(2, HD, NQC, SC)
    cs = np.ascontiguousarray(cs.transpose(2, 0, 1, 3), dtype=np.float32)
    ptT = _rope_perm_T()
    ones = np.ones((P, P), dtype=np.float32)

    # x chunk-tiled: xt[sc, ki, ko, s_local]
    xts = []
    for b in range(B):
        xT = x[b].T.reshape(KO, P, NQC, SC)
        xts.append(_bf16(xT.transpose(2, 1, 0, 3)))

    in_maps = []
    for c in range(N_CORES):
        b = c // (N_CORES // B)
        hg = c % (N_CORES // B)
        # qkv packed per head: [q_h0|k_h0|...|q_h3|k_h3|v_h0..v_h3],
        # laid out [ko, ki, col]
        cols = []
        for h in range(H_LOC):
            hh = hg * H_LOC + h
            cols.append(wq[hh * HD : (hh + 1) * HD].T)
            cols.append(wk[hh * HD : (hh + 1) * HD].T)
        for h in range(H_LOC):
            hh = hg * H_LOC + h
            cols.append(wv[hh * HD : (hh + 1) * HD].T)
        wpack = np.concatenate(cols, axis=1)  # [D, 1536]
        wpack = wpack.reshape(KO, P, QKV_W)
        wqk_h = np.ascontiguousarray(wpack[:, :, : 2 * H_LOC * HD])
        wv_h = np.ascontiguousarray(
            wpack[:, :, 2 * H_LOC * HD :].transpose(1, 0, 2)
        )
        # wo rows for this head group, laid out [ki, h, d]
        rows = slice(hg * H_LOC * HD, (hg + 1) * H_LOC * HD)
        wot = wo[:, rows].T.reshape(H_LOC, P, D)
        wot = np.ascontiguousarray(wot.transpose(1, 0, 2))
        in_maps.append(
            {
                "xt": xts[b],
                "wqk": _bf16(wqk_h),
                "wv": _bf16(wv_h),
                "wo": _bf16(wot),
                "cs": cs,
                "pt": ptT,
                "ones": _bf16(ones),
                "mask": mask_h,
            }
        )

    kw = {}
    if _trace:
        kw = dict(trace=True)
    res = run_bass_kernel_spmd(
        nc, in_maps, core_ids=list(range(N_CORES)), **kw
    )
    global last_exec_time_ns, last_profile_json
    last_exec_time_ns = res.exec_time_ns
    last_profile_json = res.profile_json

    out = np.empty((B, S, D), dtype=np.float32)
    gs = N_CORES // B
    for b in range(B):
        acc = res.results[b * gs]["y"].astype(np.float32)
        for g in range(1, gs):
            acc += res.results[b * gs + g]["y"].astype(np.float32)
        out[b] = acc
    return out


# revision 19
# speedup vs baseline: 1.1999x; 1.0626x over previous
"""Trainium2 8-core kernel for nn_Attention_27530740367526.

Multi-head causal attention (B=2, S=2048, D=2048, H=16, HD=128, fp32) with
RoPE, sharded batch x head-group across 8 NeuronCores: core c handles batch
c//4 and heads [4*(c%4), 4*(c%4)+4).  Each core computes q/k/v projections
(+RoPE), attention for its 4 heads, and the slice of the wo projection those
heads feed — a partial [S, D] output.  The host sums the 4 partials per
batch (the row-parallel wo "all-reduce" is a host-side unshard).

Single fused pass: per 512-column sequence chunk (causal order) the kernel
projects q/k/v for all 4 local heads, runs attention for the chunk's queries
(head pairs interleaved so the PE always has two independent softmax chains),
and the previous chunk's wo projection (all 4 heads accumulated in PSUM, one
bf16 output write) drains into the attention's softmax-wait bubbles.

All matmul operands are bf16 (fast weight loads, half the DMA/SBUF), with
fp32 PSUM accumulation; the RoPE rotate-half runs as a f32r 128x128
permutation matmul on the PE.  Scores live in "transposed land" ([k, q] with
head-dim contraction) so softmax denominators come from an all-ones matmul
and PV/wo consume natural layouts with zero on-device transposes.  Diagonal
score tiles are narrowed to skip fully-masked columns.  Every DRAM tensor is
host-pre-tiled so each DMA descriptor is contiguous per partition.
"""

import sys

if "/opt/trn_rl_repo" not in sys.path:
    sys.path.insert(0, "/opt/trn_rl_repo")

from collections import deque

import numpy as np
import ml_dtypes

import concourse.bacc as bacc
import concourse.mybir as mybir
import concourse.tile as tile
from concourse.bass_utils import run_bass_kernel_spmd

F32 = mybir.dt.float32
F32R = mybir.dt.float32r
BF16 = mybir.dt.bfloat16
AF = mybir.ActivationFunctionType

N_HEADS = 16
N_CORES = 8
B, S, D = 2, 2048, 2048
HD = D // N_HEADS
H_LOC = N_HEADS // (N_CORES // B)  # 4 heads per core
SC = 512                           # seq chunk (matmul moving free dim)
P = 128
KO = D // P                        # 16 contraction subtiles for projections
NQC = S // SC                      # 4 q-chunks
NSUB = SC // P                     # 4 128-blocks per chunk
NST = S // P                       # 16 s-tiles
QKV_W = 3 * H_LOC * HD             # 1536 packed qkv columns
LOOKAHEAD = 3                      # scores-tile software pipeline depth


def _build_core_kernel():
    inv_sqrt_hd = 1.0 / float(np.sqrt(HD))

    nc = bacc.Bacc(None, target_bir_lowering=False)

    # host-pre-tiled inputs: every slice below is contiguous per partition
    xt_d = nc.dram_tensor("xt", [NQC, P, KO, SC], BF16, kind="ExternalInput")
    wqk_d = nc.dram_tensor(
        "wqk", [KO, P, 2 * H_LOC * HD], BF16, kind="ExternalInput"
    )
    wv_d = nc.dram_tensor(
        "wv", [P, KO, H_LOC * HD], BF16, kind="ExternalInput"
    )
    wo_d = nc.dram_tensor("wo", [P, H_LOC, D], BF16, kind="ExternalInput")
    cs_d = nc.dram_tensor("cs", [NQC, 2, P, SC], F32, kind="ExternalInput")
    pt_d = nc.dram_tensor("pt", [P, HD], F32R, kind="ExternalInput")
    ones_d = nc.dram_tensor("ones", [P, P], BF16, kind="ExternalInput")
    mask_d = nc.dram_tensor("mask", [P, NSUB, P], F32, kind="ExternalInput")
    y = nc.dram_tensor("y", [S, D], BF16, kind="ExternalOutput")

    with tile.TileContext(nc) as tc:
        with (
            tc.tile_pool(name="persist", bufs=1) as persist,
            tc.tile_pool(name="xa", bufs=2) as xa,
            tc.tile_pool(name="cs", bufs=2) as cspool,
            tc.tile_pool(name="scr", bufs=2) as scr,
            tc.tile_pool(name="exps", bufs=4) as expp,
            tc.tile_pool(name="outq", bufs=2) as outqp,
            tc.tile_pool(name="yo", bufs=4) as yop,
            tc.tile_pool(name="ps", bufs=3, space="PSUM") as cyc,
            tc.tile_pool(name="ops", bufs=2, space="PSUM") as ops,
            tc.tile_pool(name="dps", bufs=2, space="PSUM") as dps,
            tc.tile_pool(name="yps", bufs=1, space="PSUM") as yps,
        ):
            # small persistent constants (scalar queue, ahead of big loads)
            pt_sb = persist.tile([P, HD], F32R)
            nc.scalar.dma_start(pt_sb[:], pt_d[:])
            ones_sb = persist.tile([P, P], BF16)
            nc.scalar.dma_start(ones_sb[:], ones_d[:])
            mask_sb = persist.tile([P, NSUB, P], F32)
            nc.scalar.dma_start(mask_sb[:], mask_d[:])

            # per-head-pair persistent k/v for the whole sequence
            kT_sb = persist.tile([P, H_LOC, S], BF16)
            v_sb = persist.tile([P, NST, H_LOC * HD], BF16)
            wqk_sb = persist.tile([P, KO, 2 * H_LOC * HD], BF16)
            wv_sb = persist.tile([P, KO, H_LOC * HD], BF16)
            wo_sb = persist.tile([P, H_LOC, D], BF16)

            def load_chunk(sc):
                # prefetched a chunk ahead: one fully-contiguous descriptor
                # on the scalar queue, which is idle of DMA work by then
                xt = xa.tile([P, KO, SC], BF16, tag="xt")
                nc.scalar.dma_start(xt[:], xt_d[sc])
                cos_t = cspool.tile([P, SC], F32, tag="cos")
                sin_t = cspool.tile([P, SC], F32, tag="sin")
                nc.scalar.dma_start(cos_t[:], cs_d[sc, 0])
                nc.scalar.dma_start(sin_t[:], cs_d[sc, 1])
                return xt, cos_t, sin_t

            # startup: both DMA rings stripe over the same 16 engines, so
            # feed the startup-critical stream alone and in consumption
            # order — (xt0[ko], wv[ko]) pairs feed the chunk-0 v
            # projections (which run first for chunk 0), wqk streams in
            # behind for the q/k groups; wo is deferred to the scalar
            # queue's post-projection emission point
            xt0 = xa.tile([P, KO, SC], BF16, tag="xt")
            for kg in range(4):
                nc.sync.dma_start(
                    xt0[:, 4 * kg : 4 * kg + 4],
                    xt_d[0, :, 4 * kg : 4 * kg + 4],
                )
                nc.sync.dma_start(
                    wv_sb[:, 4 * kg : 4 * kg + 4],
                    wv_d[:, 4 * kg : 4 * kg + 4],
                )
            cos_0 = cspool.tile([P, SC], F32, tag="cos")
            sin_0 = cspool.tile([P, SC], F32, tag="sin")
            nc.sync.dma_start(cos_0[:], cs_d[0, 0])
            nc.sync.dma_start(sin_0[:], cs_d[0, 1])
            preloaded = (xt0, cos_0, sin_0)
            for ko in range(KO):
                nc.sync.dma_start(wqk_sb[:, ko], wqk_d[ko])

            # dummy matmuls: trip the PE HAM clock-gate to full rate while
            # the first (xt0, wv) slices stream in.  They write the
            # (startup-idle) y PSUM bank; the bufs=1 WAW chain serializes
            # them on the PE only.
            def warmup(n):
                for wu in range(n):
                    wps = yps.tile([P, SC], F32, tag="y")
                    nc.tensor.matmul(
                        wps[:, :P], ones_sb[:], ones_sb[:],
                        skip_group_check=True,
                    )

            warmup(16)

            def project_chunk(sc, loaded):
                ssl = slice(sc * SC, (sc + 1) * SC)
                xt, cos_t, sin_t = loaded
                qT_c = outqp.tile([P, H_LOC, SC], BF16, tag="qTc")

                def vproj():
                    for sti in range(NSUB):
                        st = sc * NSUB + sti
                        lsl = slice(sti * P, (sti + 1) * P)
                        psv = cyc.tile([P, H_LOC * HD], F32, tag="ps")
                        for ko in range(KO):
                            nc.tensor.matmul(
                                psv[:],
                                xt[:, ko, lsl],
                                wv_sb[:, ko],
                                start=(ko == 0),
                                stop=(ko == KO - 1),
                            )
                        nc.scalar.copy(v_sb[:, st, :], psv[:])

                def qkproj():
                    for h in range(H_LOC):
                        for t in range(2):  # 0=q, 1=k
                            wcols = slice(
                                (2 * h + t) * HD, (2 * h + t + 1) * HD
                            )
                            ps = cyc.tile([P, SC], F32, tag="ps")
                            for ko in range(KO):
                                nc.tensor.matmul(
                                    ps[:],
                                    wqk_sb[:, ko, wcols],
                                    xt[:, ko],
                                    start=(ko == 0),
                                    stop=(ko == KO - 1),
                                )
                            plain = scr.tile([P, SC], F32R, tag="plain")
                            nc.scalar.copy(plain[:], ps[:])
                            rot = cyc.tile([P, SC], F32, tag="ps")
                            nc.tensor.matmul(rot[:], pt_sb[:], plain[:])
                            dst = (
                                qT_c[:, h, :] if t == 0 else kT_sb[:, h, ssl]
                            )
                            # rope: dst = plain*cos + rot*sin
                            pc = scr.tile([P, SC], F32, tag="pc")
                            nc.gpsimd.tensor_mul(pc[:], plain[:], cos_t[:])
                            tmp2 = scr.tile([P, SC], F32, tag="tmp2")
                            nc.vector.tensor_mul(tmp2[:], rot[:], sin_t[:])
                            nc.vector.tensor_add(dst, pc[:], tmp2[:])

                if sc == 0:
                    # chunk 0 is fed by the startup stream in (xt, wv),
                    # then wqk order — consume in that order
                    vproj()
                    qkproj()
                else:
                    qkproj()
                    vproj()
                return qT_c

            def attend_pair(qc, qT_c, hp, outT_qc, fillers):
                """Attention for query chunk qc, heads (2hp, 2hp+1)
                interleaved per k-block (so the PE always has two
                independent softmax chains in flight), writing normalized
                outT [hd, q] slices.  `fillers` is a deque of independent
                PE-work closures drained into the pipeline's tail bubbles.
                Diagonal k-blocks are narrowed to their live q columns."""
                nkb = (qc + 1) * NSUB
                qt = {}
                o_ps = {}
                d_ps = {}
                for hl in range(2):
                    h = 2 * hp + hl
                    qt[hl] = qT_c[:, h, :]
                    o_ps[hl] = ops.tile([P, SC], F32, tag="o", name=f"o_ps{hl}")
                    d_ps[hl] = dps.tile([P, SC], F32, tag="d", name=f"d_ps{hl}")
                stile = {}

                def q0(kb):
                    # first live q column for k-block kb (causal narrowing)
                    j = kb - qc * NSUB
                    return j * P if j > 0 else 0

                def emit_scores(kb, hl):
                    h = 2 * hp + hl
                    c0 = q0(kb)
                    t_ = cyc.tile([P, SC], F32, tag="ps")
                    nc.tensor.matmul(
                        t_[:, c0:],
                        kT_sb[:, h, kb * P : (kb + 1) * P],
                        qt[hl][:, c0:],
                        skip_group_check=True,
                    )
                    j = kb - qc * NSUB
                    if j >= 0:
                        # triangular boundary block only
                        nc.vector.tensor_add(
                            t_[:, c0 : c0 + P],
                            t_[:, c0 : c0 + P],
                            mask_sb[:, j, :],
                        )
                    stile[(kb, hl)] = t_

                seq = [(kb, hl) for kb in range(nkb) for hl in range(2)]
                for kb, hl in seq[:LOOKAHEAD]:
                    emit_scores(kb, hl)
                for i, (kb, hl) in enumerate(seq):
                    c0 = q0(kb)
                    h = 2 * hp + hl
                    e = expp.tile([P, SC], BF16, tag="e")
                    nc.scalar.activation(
                        e[:, c0:], stile.pop((kb, hl))[:, c0:], AF.Exp,
                        scale=inv_sqrt_hd,
                    )
                    nc.tensor.matmul(
                        o_ps[hl][:, c0:],
                        v_sb[:, kb, h * HD : (h + 1) * HD],
                        e[:, c0:],
                        start=(kb == 0),
                        stop=(kb == nkb - 1),
                        skip_group_check=True,
                    )
                    nc.tensor.matmul(
                        d_ps[hl][:, c0:],
                        ones_sb[:],
                        e[:, c0:],
                        start=(kb == 0),
                        stop=(kb == nkb - 1),
                        skip_group_check=True,
                    )
                    if i + LOOKAHEAD < len(seq):
                        emit_scores(*seq[i + LOOKAHEAD])
                        if fillers and i % 2 == 1:
                            fillers.popleft()()
                    elif fillers:
                        fillers.popleft()()
                for hl in range(2):
                    h = 2 * hp + hl
                    recip = scr.tile([P, SC], F32, tag="recip")
                    nc.vector.reciprocal_approx_fast(recip[:], d_ps[hl][:])
                    nc.vector.tensor_mul(
                        outT_qc[:, h, :], o_ps[hl][:], recip[:]
                    )

            def make_out_fillers(qc, outT_qc, tail=False):
                """One closure per (s-tile, d-chunk) block of the wo
                projection for query chunk qc: 4 accumulating matmuls (all
                local heads), a PSUM->SBUF bf16 copy, and the output DMA.
                Tail blocks (after the last attend) ping-pong across the
                now-idle attention PSUM banks and both copy engines."""
                work = []
                for sti in range(NSUB):
                    st = qc * NSUB + sti
                    stsl = slice(sti * P, (sti + 1) * P)
                    for dc in range(D // SC):
                        dsl = slice(dc * SC, (dc + 1) * SC)
                        bi = sti * (D // SC) + dc

                        def blk(st=st, stsl=stsl, dsl=dsl, bi=bi):
                            if tail:
                                pool, tag = [
                                    (yps, "y"), (dps, "d"), (ops, "o")
                                ][bi % 3]
                                y_ps = pool.tile([P, SC], F32, tag=tag)
                            else:
                                y_ps = yps.tile([P, SC], F32, tag="y")
                            for h in range(H_LOC):
                                nc.tensor.matmul(
                                    y_ps[:],
                                    outT_qc[:, h, stsl],
                                    wo_sb[:, h, dsl],
                                    start=(h == 0),
                                    stop=(h == H_LOC - 1),
                                )
                            y_sb = yop.tile([P, SC], BF16, tag="ysb")
                            if tail and bi % 2 == 1:
                                nc.scalar.copy(y_sb[:], y_ps[:])
                            else:
                                nc.vector.tensor_copy(y_sb[:], y_ps[:])
                            nc.sync.dma_start(
                                y[st * P : (st + 1) * P, dsl], y_sb[:]
                            )

                        work.append(blk)
                return work

            pending = deque()
            for sc in range(NQC):
                loaded = preloaded if sc == 0 else load_chunk(sc)
                preloaded = None
                qT_c = project_chunk(sc, loaded)
                if sc == 0:
                    # wo lands on the scalar queue only after chunk 0's
                    # projection copies — keeps the startup stream alone on
                    # the shared DMA engines (first needed at ~65us)
                    for h in range(H_LOC):
                        nc.scalar.dma_start(wo_sb[:, h], wo_d[:, h])
                outT_qc = outqp.tile([P, H_LOC, SC], BF16, tag="outq")
                for hp in range(2):
                    attend_pair(sc, qT_c, hp, outT_qc, pending)
                pending.extend(
                    make_out_fillers(sc, outT_qc, tail=(sc == NQC - 1))
                )
            while pending:
                pending.popleft()()

    nc.compile()
    return nc


_NC_CACHE = {}


def _get_nc():
    if "nc" not in _NC_CACHE:
        _NC_CACHE["nc"] = _build_core_kernel()
    return _NC_CACHE["nc"]


def _rope_perm_T() -> np.ndarray:
    # rotate_half as a matrix: (P_rh @ q)[d] = -q[d+HD/2] for d < HD/2,
    # q[d-HD/2] otherwise.  Returns P_rh.T for use as matmul lhsT.
    P_rh = np.zeros((HD, HD), dtype=np.float32)
    half = HD // 2
    for i in range(half):
        P_rh[i, half + i] = -1.0
        P_rh[half + i, i] = 1.0
    return np.ascontiguousarray(P_rh.T)


def _is_causal(m: np.ndarray) -> bool:
    tril = np.tril(np.ones((S, S), dtype=bool))
    if not np.all(m[tril] == 0.0):
        return False
    upper = m[~tril]
    return bool(upper.size == 0 or np.all(upper <= -1.0e8))


def _bf16(x: np.ndarray) -> np.ndarray:
    return np.ascontiguousarray(x).astype(ml_dtypes.bfloat16)


def _reference_numpy(x, cos, sin, mask, wq, wk, wv, wo):
    # generic-mask fallback (never hit for the causal reference mask)
    def rot_half(t):
        t1, t2 = np.split(t, 2, axis=-1)
        return np.concatenate((-t2, t1), axis=-1)

    H = N_HEADS
    q = (x @ wq.T).reshape(B, S, H, HD).transpose(0, 2, 1, 3)
    k = (x @ wk.T).reshape(B, S, H, HD).transpose(0, 2, 1, 3)
    v = (x @ wv.T).reshape(B, S, H, HD).transpose(0, 2, 1, 3)
    c = cos[None, None]
    s = sin[None, None]
    q = q * c + rot_half(q) * s
    k = k * c + rot_half(k) * s
    scores = np.einsum("bhqd,bhkd->bhqk", q, k) / np.sqrt(np.float32(HD))
    scores = scores + mask
    scores -= scores.max(axis=-1, keepdims=True)
    p = np.exp(scores)
    p /= p.sum(axis=-1, keepdims=True)
    out = np.einsum("bhqk,bhkd->bhqd", p, v)
    out = out.transpose(0, 2, 1, 3).reshape(B, S, D)
    return (out @ wo.T).astype(np.float32)


# module-level: results of the last traced run (for test harnesses)
last_exec_time_ns = None
last_profile_json = None


def kernel(x, cos, sin, mask, wq, wk, wv, wo, _trace=False):
    x = np.asarray(x, dtype=np.float32)
    cos = np.asarray(cos, dtype=np.float32)
    sin = np.asarray(sin, dtype=np.float32)
    mask = np.asarray(mask, dtype=np.float32)
    wq = np.asarray(wq, dtype=np.float32)
    wk = np.asarray(wk, dtype=np.float32)
    wv = np.asarray(wv, dtype=np.float32)
    wo = np.asarray(wo, dtype=np.float32)

    m2d = mask.reshape(S, S)
    if not _is_causal(m2d):
        return _reference_numpy(x, cos, sin, mask, wq, wk, wv, wo)
    nc = _get_nc()

    scale = np.float32(np.sqrt(HD))
    # [k, q] transposed causal boundary blocks: mask_h[ki, j, q_local]
    mt = np.ascontiguousarray((m2d[:SC, :SC] * scale).T).reshape(NSUB, P, NSUB, P)
    mask_h = np.ascontiguousarray(
        np.stack([mt[j, :, j, :] for j in range(NSUB)], axis=1)
    )
    # cos/sin chunk-tiled: cs[sc, {cos,sin}, hd, s_local]
    cs = np.stack([cos.T, sin.T], axis=0).reshape(2, HD, NQC, SC)
    cs = np.ascontiguousarray(cs.transpose(2, 0, 1, 3), dtype=np.float32)
    ptT = _rope_perm_T()
    ones = np.ones((P, P), dtype=np.float32)

    # x chunk-tiled: xt[sc, ki, ko, s_local]
    xts = []
    for b in range(B):
        xT = x[b].T.reshape(KO, P, NQC, SC)
        xts.append(_bf16(xT.transpose(2, 1, 0, 3)))

    in_maps = []
    for c in range(N_CORES):
        b = c // (N_CORES // B)
        hg = c % (N_CORES // B)
        # qkv packed per head: [q_h0|k_h0|...|q_h3|k_h3|v_h0..v_h3],
        # laid out [ko, ki, col]
        cols = []
        for h in range(H_LOC):
            hh = hg * H_LOC + h
            cols.append(wq[hh * HD : (hh + 1) * HD].T)
            cols.append(wk[hh * HD : (hh + 1) * HD].T)
        for h in range(H_LOC):
            hh = hg * H_LOC + h
            cols.append(wv[hh * HD : (hh + 1) * HD].T)
        wpack = np.concatenate(cols, axis=1)  # [D, 1536]
        wpack = wpack.reshape(KO, P, QKV_W)
        wqk_h = np.ascontiguousarray(wpack[:, :, : 2 * H_LOC * HD])
        wv_h = np.ascontiguousarray(
            wpack[:, :, 2 * H_LOC * HD :].transpose(1, 0, 2)
        )
        # wo rows for this head group, laid out [ki, h, d]
        rows = slice(hg * H_LOC * HD, (hg + 1) * H_LOC * HD)
        wot = wo[:, rows].T.reshape(H_LOC, P, D)
        wot = np.ascontiguousarray(wot.transpose(1, 0, 2))
        in_maps.append(
            {
                "xt": xts[b],
                "wqk": _bf16(wqk_h),
                "wv": _bf16(wv_h),
                "wo": _bf16(wot),
                "cs": cs,
                "pt": ptT,
                "ones": _bf16(ones),
                "mask": mask_h,
            }
        )

    kw = {}
    if _trace:
        kw = dict(trace=True)
    res = run_bass_kernel_spmd(
        nc, in_maps, core_ids=list(range(N_CORES)), **kw
    )
    global last_exec_time_ns, last_profile_json
    last_exec_time_ns = res.exec_time_ns
    last_profile_json = res.profile_json

    out = np.empty((B, S, D), dtype=np.float32)
    gs = N_CORES // B
    for b in range(B):
        acc = res.results[b * gs]["y"].astype(np.float32)
        for g in range(1, gs):
            acc += res.results[b * gs + g]["y"].astype(np.float32)
        out[b] = acc
    return out
